# revision 1
# baseline (speedup 1.0000x reference)
"""DIFFormerConv (simple linear attention + dense GCN) on 8 trn2 NeuronCores.

Sharding: nodes N=4096 split 8 ways (S=512 per core). Each core computes
q/k/v for its node shard, partial kvs/ks_sum/vsum (AllReduce), vmean
(AllGather, bf16), the attention output rows for its shard, and the GCN
rows for its shard (adj^T column shard, bf16 matmul).

Layouts chosen so no PE transposes are needed:
  q:   [hd, s]  (heads*dim on partitions)   -- lhsT = W^T chunks
  k,v: [s, hd]  (transposed projection)     -- lhsT = x chunks
  gcn: [(b,d), n] directly                  -- lhsT = vmean[m,(b,d)], rhs = adjT[m,n]
Host prep: adjT = adj.T + I (bf16), rrs = 0.25/(rowsum+1), W transposes.
"""

import sys

sys.path.insert(0, "/opt/trn_rl_repo")

import numpy as np
import ml_dtypes

from concourse import bass, bacc, tile, mybir
from concourse.bass_utils import run_bass_kernel_spmd

B, C, N, H, D = 8, 256, 4096, 4, 64
NCORES = 8
S = N // NCORES          # 512 nodes per core
HD = H * D               # 256
F32 = mybir.dt.float32
F32R = mybir.dt.float32r
BF16 = mybir.dt.bfloat16
AX = mybir.AxisListType.X
ALU = mybir.AluOpType
ACTF = mybir.ActivationFunctionType
RG = [list(range(NCORES))]

_CACHE = {}
DEBUG_DUMPS = False


def _indicators():
    i4a = np.zeros((128, 4), np.float32)
    i4b = np.zeros((128, 4), np.float32)
    for p in range(128):
        i4a[p, p // 64] = 1.0
        i4b[p, 2 + p // 64] = 1.0
    ibc0 = np.zeros((4, 128), np.float32)
    ibc1 = np.zeros((4, 128), np.float32)
    for p in range(128):
        ibc0[p // 64, p] = 1.0
        ibc1[2 + p // 64, p] = 1.0
    return i4a, i4b, ibc0, ibc1


def _build():
    nc = bacc.Bacc("TRN2", target_bir_lowering=False, debug=False,
                   num_devices=NCORES)

    xq = nc.dram_tensor("xq", [B, 2, 128, S], F32R, kind="ExternalInput")
    xs = nc.dram_tensor("xs", [B, 2, 128, S], F32R, kind="ExternalInput")
    adjt = nc.dram_tensor("adjt", [32, 128, S], BF16, kind="ExternalInput")
    rrs = nc.dram_tensor("rrs", [1, S], F32R, kind="ExternalInput")
    wqt = nc.dram_tensor("wqt", [2, 128, HD], F32R, kind="ExternalInput")
    wkt = nc.dram_tensor("wkt", [2, 128, HD], F32R, kind="ExternalInput")
    wvt = nc.dram_tensor("wvt", [2, 128, HD], F32R, kind="ExternalInput")
    bqr = nc.dram_tensor("bqr", [1, HD], F32R, kind="ExternalInput")
    bkr = nc.dram_tensor("bkr", [1, HD], F32R, kind="ExternalInput")
    bvr = nc.dram_tensor("bvr", [1, HD], F32R, kind="ExternalInput")
    out = nc.dram_tensor("out", [B, D, S], F32, kind="ExternalOutput")
    if DEBUG_DUMPS:
        dbg_ar = nc.dram_tensor("dbg_ar", [B, 2, 132, D], F32,
                                kind="ExternalOutput")
        dbg_vm = nc.dram_tensor("dbg_vm", [NCORES, S, B, D], BF16,
                                kind="ExternalOutput")

    i4a_d = nc.dram_tensor("i4a_in", [128, 4], F32R, kind="ExternalInput")
    i4b_d = nc.dram_tensor("i4b_in", [128, 4], F32R, kind="ExternalInput")
    ibc0_d = nc.dram_tensor("ibc0_in", [4, 128], F32R, kind="ExternalInput")
    ibc1_d = nc.dram_tensor("ibc1_in", [4, 128], F32R, kind="ExternalInput")
    ones_r_d = nc.dram_tensor("ones_r", [1, S], F32R, kind="ExternalInput")
    ones_c_d = nc.dram_tensor("ones_c", [128, 1], F32R, kind="ExternalInput")

    def r(ap):
        return ap

    with nc.allow_low_precision(reason="float32r rounding intentional"), \
            tile.TileContext(nc) as tc:
        with (
            tc.tile_pool(name="pers", bufs=1) as pp,
            tc.tile_pool(name="work", bufs=3) as wk,
            tc.tile_pool(name="dram", bufs=1, space="DRAM") as dp,
        ):
            # DRAM internal buffers for collectives
            vm_loc = dp.tile([S, B, D], BF16, tag="vm_loc", name="vm_loc")
            vm_all = dp.tile([NCORES, S, B, D], BF16, tag="vm_all", name="vm_all", addr_space="Shared")
            ar_in = dp.tile([B, 2, 132, D], F32, tag="ar_in", name="ar_in")
            ar_out = dp.tile([B, 2, 132, D], F32, tag="ar_out", name="ar_out", addr_space="Shared")

            # ---- constants ----
            wq_t = [pp.tile([128, HD], F32R, tag=f"wq{c}", name=f"wq{c}") for c in range(2)]
            wk_t = [pp.tile([128, HD], F32R, tag=f"wk{c}", name=f"wk{c}") for c in range(2)]
            wv_t = [pp.tile([128, HD], F32R, tag=f"wv{c}", name=f"wv{c}") for c in range(2)]
            for c in range(2):
                nc.sync.dma_start(out=wq_t[c][:], in_=wqt[c])
                nc.sync.dma_start(out=wk_t[c][:], in_=wkt[c])
                nc.sync.dma_start(out=wv_t[c][:], in_=wvt[c])
            bq_row = pp.tile([1, HD], F32R, tag="bqrow")
            bk_row = pp.tile([1, HD], F32R, tag="bkrow")
            bv_row = pp.tile([1, HD], F32R, tag="bvrow")
            nc.sync.dma_start(out=bq_row[:], in_=bqr[:])
            nc.sync.dma_start(out=bk_row[:], in_=bkr[:])
            nc.sync.dma_start(out=bv_row[:], in_=bvr[:])
            i4a = pp.tile([128, 4], F32R, tag="i4a")
            i4b = pp.tile([128, 4], F32R, tag="i4b")
            ibc0 = pp.tile([4, 128], F32R, tag="ibc0")
            ibc1 = pp.tile([4, 128], F32R, tag="ibc1")
            nc.sync.dma_start(out=i4a[:], in_=i4a_d[:])
            nc.sync.dma_start(out=i4b[:], in_=i4b_d[:])
            nc.sync.dma_start(out=ibc0[:], in_=ibc0_d[:])
            nc.sync.dma_start(out=ibc1[:], in_=ibc1_d[:])
            ones_row = pp.tile([1, S], F32R, tag="ones_row")
            ones_col = pp.tile([128, 1], F32R, tag="ones_col")
            nc.sync.dma_start(out=ones_row[:], in_=ones_r_d[:])
            nc.sync.dma_start(out=ones_col[:], in_=ones_c_d[:])
            rrs_row = pp.tile([1, S], F32R, tag="rrs_row")
            nc.sync.dma_start(out=rrs_row[:], in_=rrs[:])

            # persistent per-batch SBUF tensors
            q_sb = [[pp.tile([128, S], F32R, tag=f"q{b}_{h}", name=f"q{b}_{h}") for h in range(2)]
                    for b in range(B)]
            kt_sb = [[pp.tile([128, HD], F32R, tag=f"kt{b}_{s}", name=f"kt{b}_{s}") for s in range(4)]
                     for b in range(B)]
            vt_sb = [[pp.tile([128, HD], F32R, tag=f"vt{b}_{s}", name=f"vt{b}_{s}") for s in range(4)]
                     for b in range(B)]
            rq_sb = [pp.tile([4, S], F32, tag=f"rq{b}", name=f"rq{b}") for b in range(B)]
            attn_sb = [pp.tile([128, S], F32, tag=f"at{p}", name=f"at{p}")
                       for p in range(4)]
            rrs_bc = pp.tile([128, S], F32, tag="rrs_bc")

            with tc.tile_pool(name="psA", bufs=1, space="PSUM") as psA:
                # broadcast rrs row to all 128 partitions (K=1 matmul)
                pbc0 = psA.tile([128, S], F32, tag="pq")
                nc.tensor.matmul(pbc0[:], lhsT=r(ones_row[:, 0:128]),
                                 rhs=r(rrs_row[:]), start=True, stop=True)
                nc.scalar.activation(rrs_bc[:], pbc0[:], ACTF.Copy)

                # =================== phase 1: per-batch local ===================
                for b in range(B):
                    xs0 = wk.tile([128, S], F32R, tag="xs0", bufs=2)
                    xs1 = wk.tile([128, S], F32R, tag="xs1", bufs=2)
                    nc.sync.dma_start(out=xs0[:], in_=xs[b, 0])
                    nc.sync.dma_start(out=xs1[:], in_=xs[b, 1])

                    kvs_ps0 = psA.tile([128, HD], F32, tag="kvs0")
                    kvs_ps1 = psA.tile([128, HD], F32, tag="kvs1")
                    ks_ps = psA.tile([1, HD], F32, tag="ksps")
                    vs_ps = psA.tile([1, HD], F32, tag="vsps")

                    for sb_i in range(4):
                        sl = slice(sb_i * 128, (sb_i + 1) * 128)
                        # k^T and v^T projections: out[s, hd]
                        pk = psA.tile([128, HD], F32, tag="pk")
                        pv = psA.tile([128, HD], F32, tag="pv")
                        for (ps, wt, brow) in ((pk, wk_t, bk_row),
                                               (pv, wv_t, bv_row)):
                            nc.tensor.matmul(ps[:], lhsT=r(xs0[:, sl]),
                                             rhs=r(wt[0][:]), start=True,
                                             stop=False)
                            nc.tensor.matmul(ps[:], lhsT=r(xs1[:, sl]),
                                             rhs=r(wt[1][:]), start=False,
                                             stop=False)
                            nc.tensor.matmul(ps[:], lhsT=r(ones_row[:, 0:128]),
                                             rhs=r(brow[:]), start=False,
                                             stop=True)
                        # v^T evac
                        nc.scalar.activation(vt_sb[b][sb_i][:], pv[:], ACTF.Copy)
                        # vmean (sum over heads; /4 folded into rrs) -> bf16
                        vm_t = wk.tile([128, D], BF16, tag="vmt")
                        with nc.allow_low_precision(reason="vmean bf16 is ok"):
                            nc.vector.reduce_sum(
                                vm_t[:], pv[:].rearrange("p (h d) -> p d h",
                                                         h=H),
                                axis=AX)
                        nc.sync.dma_start(out=vm_loc[sl, b, :], in_=vm_t[:])
                        # kn = k / ||k||  (per head, free-dim blocks of 64)
                        sq = wk.tile([128, HD], F32, tag="sq")
                        nc.scalar.activation(sq[:], pk[:], ACTF.Square)
                        ssk = wk.tile([128, H], F32, tag="ssk")
                        nc.vector.reduce_sum(
                            ssk[:], sq[:].rearrange("p (h d) -> p h d", h=H),
                            axis=AX)
                        snk = wk.tile([128, H], F32, tag="snk")
                        nc.scalar.activation(snk[:], ssk[:], ACTF.Sqrt)
                        rk = wk.tile([128, H], F32, tag="rk")
                        nc.vector.reciprocal(rk[:], snk[:])
                        for h in range(H):
                            dsl = slice(h * D, (h + 1) * D)
                            nc.vector.tensor_scalar_mul(
                                kt_sb[b][sb_i][:, dsl], pk[:, dsl],
                                rk[:, h:h + 1])

                    # kvs / ks_sum / vsum partials, one contiguous
                    # accumulation group per bank
                    for sb_i in range(4):
                        nc.tensor.matmul(kvs_ps0[:],
                                         lhsT=r(kt_sb[b][sb_i][:, 0:128]),
                                         rhs=r(vt_sb[b][sb_i][:]),
                                         start=(sb_i == 0), stop=(sb_i == 3))
                    for sb_i in range(4):
                        nc.tensor.matmul(kvs_ps1[:],
                                         lhsT=r(kt_sb[b][sb_i][:, 128:HD]),
                                         rhs=r(vt_sb[b][sb_i][:]),
                                         start=(sb_i == 0), stop=(sb_i == 3))
                    for sb_i in range(4):
                        nc.tensor.matmul(ks_ps[:], lhsT=r(ones_col[:]),
                                         rhs=r(kt_sb[b][sb_i][:]),
                                         start=(sb_i == 0), stop=(sb_i == 3))
                    for sb_i in range(4):
                        nc.tensor.matmul(vs_ps[:], lhsT=r(ones_col[:]),
                                         rhs=r(vt_sb[b][sb_i][:]),
                                         start=(sb_i == 0), stop=(sb_i == 3))

                    # evac kvs diag blocks (stacked [128,(h,m) x 64 d])
                    pk0 = wk.tile([128, D], F32, tag="arpk0")
                    pk1 = wk.tile([128, D], F32, tag="arpk1")
                    nc.scalar.activation(pk0[0:64, :], kvs_ps0[0:64, 0:64],
                                         ACTF.Copy)
                    nc.scalar.activation(pk0[64:128, :],
                                         kvs_ps0[64:128, 64:128], ACTF.Copy)
                    nc.scalar.activation(pk1[0:64, :], kvs_ps1[0:64, 128:192],
                                         ACTF.Copy)
                    nc.scalar.activation(pk1[64:128, :],
                                         kvs_ps1[64:128, 192:256], ACTF.Copy)
                    ksvs_sb = wk.tile([1, 2 * HD], F32, tag="ksvs_sb", bufs=2)
                    nc.scalar.activation(ksvs_sb[0:1, 0:HD], ks_ps[:],
                                         ACTF.Copy)
                    nc.scalar.activation(ksvs_sb[0:1, HD:2 * HD], vs_ps[:],
                                         ACTF.Copy)
                    nc.sync.dma_start(out=ar_in[b, 0, 0:128, :], in_=pk0[:])
                    nc.sync.dma_start(out=ar_in[b, 1, 0:128, :], in_=pk1[:])
                    nc.sync.dma_start(out=ar_in[b, 0, 128:130, :],
                                      in_=ksvs_sb[0:1, 0:128])
                    nc.sync.dma_start(out=ar_in[b, 1, 128:130, :],
                                      in_=ksvs_sb[0:1, 128:256])
                    nc.sync.dma_start(out=ar_in[b, 0, 130:132, :],
                                      in_=ksvs_sb[0:1, 256:384])
                    nc.sync.dma_start(out=ar_in[b, 1, 130:132, :],
                                      in_=ksvs_sb[0:1, 384:512])

                    # q projection: out[hd, s]
                    xq0 = wk.tile([128, S], F32R, tag="xs0", bufs=2)
                    xq1 = wk.tile([128, S], F32R, tag="xs1", bufs=2)
                    nc.sync.dma_start(out=xq0[:], in_=xq[b, 0])
                    nc.sync.dma_start(out=xq1[:], in_=xq[b, 1])
                    ss_ps = psA.tile([4, S], F32, tag="ss")
                    for h in range(2):
                        hsl = slice(h * 128, (h + 1) * 128)
                        pq = psA.tile([128, S], F32, tag="pq")
                        nc.tensor.matmul(pq[:], lhsT=r(wq_t[0][:, hsl]),
                                         rhs=r(xq0[:]), start=True, stop=False)
                        nc.tensor.matmul(pq[:], lhsT=r(wq_t[1][:, hsl]),
                                         rhs=r(xq1[:]), start=False, stop=False)
                        nc.tensor.matmul(pq[:], lhsT=r(bq_row[:, hsl]),
                                         rhs=r(ones_row[:]), start=False,
                                         stop=True)
                        nc.scalar.activation(q_sb[b][h][:], pq[:], ACTF.Copy)
                        qsq = wk.tile([128, S], F32R, tag="qsq", bufs=2)
                        nc.scalar.activation(qsq[:], pq[:], ACTF.Square)
                        nc.tensor.matmul(ss_ps[:],
                                         lhsT=r(i4a[:] if h == 0 else i4b[:]),
                                         rhs=r(qsq[:]), start=(h == 0),
                                         stop=(h == 1))
                    snq = wk.tile([4, S], F32, tag="snq", bufs=1)
                    nc.scalar.activation(snq[:], ss_ps[:], ACTF.Sqrt)
                    nc.vector.reciprocal(rq_sb[b][:], snq[:])

            # =================== collectives ===================
            nc.gpsimd.collective_compute(
                "AllGather", ALU.bypass, ins=[vm_loc.opt()],
                outs=[vm_all.opt()], replica_groups=RG)
            nc.gpsimd.collective_compute(
                "AllReduce", ALU.add, ins=[ar_in.opt()],
                outs=[ar_out.opt()], replica_groups=RG)

            # =================== phase 2: attention epilogue ===================
            with tc.tile_pool(name="psB", bufs=2, space="PSUM") as psB:
                for b in range(B):
                    kpk0f = wk.tile([128, D], F32, tag="kpk0f")
                    kpk1f = wk.tile([128, D], F32, tag="kpk1f")
                    nc.sync.dma_start(out=kpk0f[:], in_=ar_out[b, 0, 0:128, :])
                    nc.sync.dma_start(out=kpk1f[:], in_=ar_out[b, 1, 0:128, :])
                    kpk0 = wk.tile([128, D], F32R, tag="kpk0")
                    kpk1 = wk.tile([128, D], F32R, tag="kpk1")
                    nc.scalar.activation(kpk0[:], kpk0f[:], ACTF.Copy)
                    nc.scalar.activation(kpk1[:], kpk1f[:], ACTF.Copy)
                    ksp0f = wk.tile([128, 4], F32, tag="ksp0f")
                    ksp1f = wk.tile([128, 4], F32, tag="ksp1f")
                    nc.vector.memset(ksp0f[:], 0.0)
                    nc.vector.memset(ksp1f[:], 0.0)
                    nc.sync.dma_start(out=ksp0f[0:64, 0:1],
                                      in_=ar_out[b, 0, 128, :])
                    nc.sync.dma_start(out=ksp0f[64:128, 1:2],
                                      in_=ar_out[b, 0, 129, :])
                    nc.sync.dma_start(out=ksp1f[0:64, 2:3],
                                      in_=ar_out[b, 1, 128, :])
                    nc.sync.dma_start(out=ksp1f[64:128, 3:4],
                                      in_=ar_out[b, 1, 129, :])
                    ksp0 = wk.tile([128, 4], F32R, tag="ksp0")
                    ksp1 = wk.tile([128, 4], F32R, tag="ksp1")
                    nc.scalar.activation(ksp0[:], ksp0f[:], ACTF.Copy)
                    nc.scalar.activation(ksp1[:], ksp1f[:], ACTF.Copy)
                    vspf = wk.tile([4, D], F32, tag="vspf")
                    nc.sync.dma_start(out=vspf[0:2, :],
                                      in_=ar_out[b, 0, 130:132, :])
                    nc.sync.dma_start(out=vspf[2:4, :],
                                      in_=ar_out[b, 1, 130:132, :])
                    vsp = wk.tile([4, D], F32R, tag="vsp")
                    nc.scalar.activation(vsp[:], vspf[:], ACTF.Copy)

                    pden = psB.tile([4, S], F32, tag="pb")
                    nc.tensor.matmul(pden[:], lhsT=r(ksp0[:]),
                                     rhs=r(q_sb[b][0][:]), start=True,
                                     stop=False)
                    nc.tensor.matmul(pden[:], lhsT=r(ksp1[:]),
                                     rhs=r(q_sb[b][1][:]), start=False,
                                     stop=True)
                    t0 = wk.tile([4, S], F32, tag="t0", bufs=1)
                    nc.vector.tensor_mul(t0[:], pden[:], rq_sb[b][:])
                    t1 = wk.tile([4, S], F32, tag="t1", bufs=1)
                    nc.vector.tensor_scalar(t1[:], t0[:], 4.0, float(4 * N),
                                            op0=ALU.mult, op1=ALU.add)
                    rp = wk.tile([4, S], F32R, tag="rp", bufs=2)
                    nc.vector.reciprocal(rp[:], t1[:])  # 0.25/denom
                    cc = wk.tile([4, S], F32R, tag="cc", bufs=2)
                    nc.vector.tensor_mul(cc[:], rp[:].bitcast(F32), rq_sb[b][:])

                    pat = psB.tile([D, S], F32, tag="pat")
                    for h in range(2):
                        pbc = psB.tile([128, S], F32, tag="pb")
                        nc.tensor.matmul(pbc[:],
                                         lhsT=r(ibc0[:] if h == 0 else ibc1[:]),
                                         rhs=r(cc[:]), start=True, stop=True)
                        qs = wk.tile([128, S], F32R, tag="qs", bufs=2)
                        nc.vector.tensor_mul(qs[:], q_sb[b][h][:].bitcast(F32), pbc[:])
                        nc.tensor.matmul(pat[:],
                                         lhsT=r(kpk0[:] if h == 0 else kpk1[:]),
                                         rhs=r(qs[:]), start=(h == 0),
                                         stop=False)
                    nc.tensor.matmul(pat[:], lhsT=r(vsp[:]), rhs=r(rp[:]),
                                     start=False, stop=True)
                    nc.scalar.activation(
                        attn_sb[b // 2][(b % 2) * D:(b % 2 + 1) * D, :],
                        pat[:], ACTF.Copy)

                # =================== phase 3: GCN ===================
                with tc.tile_pool(name="psC", bufs=1, space="PSUM") as psC:
                    pg = [psC.tile([128, S], F32, tag=f"g{p}", name=f"g{p}") for p in range(4)]
                    for mc in range(32):
                        adj_t = wk.tile([128, S], BF16, tag="adj")
                        nc.sync.dma_start(out=adj_t[:], in_=adjt[mc])
                        for p in range(4):
                            vm_t = wk.tile([128, 128], BF16, tag="vml")
                            lc = mc % 4
                            nc.sync.dma_start(
                                out=vm_t[:],
                                in_=vm_all[mc // 4,
                                           lc * 128:(lc + 1) * 128,
                                           2 * p:2 * p + 2, :])
                            nc.tensor.matmul(pg[p][:], lhsT=vm_t[:],
                                             rhs=adj_t[:], start=(mc == 0),
                                             stop=(mc == 31))
                    for p in range(4):
                        gt = wk.tile([128, S], F32, tag="gt", bufs=2)
                        nc.vector.tensor_mul(gt[:], pg[p][:], rrs_bc[:])
                        ot = wk.tile([128, S], F32, tag="ot", bufs=2)
                        nc.vector.tensor_add(ot[:], gt[:], attn_sb[p][:])
                        nc.sync.dma_start(out=out[2 * p], in_=ot[0:D, :])
                        nc.sync.dma_start(out=out[2 * p + 1], in_=ot[D:128, :])
                    if DEBUG_DUMPS:
                        nc.sync.dma_start(out=dbg_ar[:], in_=ar_out[:])
                        nc.sync.dma_start(out=dbg_vm[:], in_=vm_all[:])
    nc.compile()
    return nc


def _prep_inputs(query_input, source_input, adj, Wq_w, Wq_b, Wk_w, Wk_b,
                 Wv_w, Wv_b):
    xq_np = np.asarray(query_input, dtype=np.float32)
    xs_np = np.asarray(source_input, dtype=np.float32)
    adj_np = np.asarray(adj, dtype=np.float32)

    adjT = np.ascontiguousarray(adj_np.T)
    np.fill_diagonal(adjT, adjT.diagonal() + 1.0)
    adjT_bf = adjT.astype(ml_dtypes.bfloat16)
    rrs_full = (0.25 / (adj_np.sum(axis=1) + 1.0)).astype(np.float32)

    wqt = np.ascontiguousarray(np.asarray(Wq_w, np.float32).T).reshape(2, 128, HD)
    wkt = np.ascontiguousarray(np.asarray(Wk_w, np.float32).T).reshape(2, 128, HD)
    wvt = np.ascontiguousarray(np.asarray(Wv_w, np.float32).T).reshape(2, 128, HD)
    bq = np.asarray(Wq_b, np.float32).reshape(1, HD)
    bk = np.asarray(Wk_b, np.float32).reshape(1, HD)
    bv = np.asarray(Wv_b, np.float32).reshape(1, HD)

    i4a, i4b, ibc0, ibc1 = _indicators()
    in_maps = []
    for i in range(NCORES):
        sl = slice(i * S, (i + 1) * S)
        in_maps.append({
            "xq": np.ascontiguousarray(xq_np[:, :, sl]).reshape(B, 2, 128, S),
            "xs": np.ascontiguousarray(xs_np[:, :, sl]).reshape(B, 2, 128, S),
            "adjt": np.ascontiguousarray(adjT_bf[:, sl]).reshape(32, 128, S),
            "rrs": np.ascontiguousarray(rrs_full[sl]).reshape(1, S),
            "wqt": wqt, "wkt": wkt, "wvt": wvt,
            "bqr": bq, "bkr": bk, "bvr": bv,
            "i4a_in": i4a, "i4b_in": i4b,
            "ibc0_in": ibc0, "ibc1_in": ibc1,
            "ones_r": np.ones((1, S), np.float32),
            "ones_c": np.ones((128, 1), np.float32),
        })
    return in_maps


def kernel(**inputs):
    if "nc" not in _CACHE:
        _CACHE["nc"] = _build()
    nc = _CACHE["nc"]
    in_maps = _prep_inputs(**inputs)
    res = run_bass_kernel_spmd(nc, in_maps, list(range(NCORES)))
    full = np.empty((B, D, N), np.float32)
    for i in range(NCORES):
        full[:, :, i * S:(i + 1) * S] = res.results[i]["out"]
    return full

